# revision 1
# baseline (speedup 1.0000x reference)
# Trainium2 Bass kernel for the CustomESN problem (8 NeuronCores).
#
# Math (reference):
#   u_t = x_t @ W_in                                  [B, R]
#   s_{t+1} = 0.5*s_t + 0.5*tanh(s_t @ W_res + u_t)   (T steps, s_0 = 0)
#   out = s_T @ W_out                                 [B, O]
#
# Substitution sigma_t = 2*s_t folds one 0.5 into pre-scaled weights:
#   sigma_{t+1} = 0.5*sigma_t + tanh(sigma_t @ (0.5*W_res) + u_t)
#   out = sigma_T @ (0.5*W_out)
# so the per-step elementwise update is one fused DVE op:
#   sigma' = (sigma * 0.5) + tanh_result
#
# Sharding: data-parallel, batch 512 -> 8 cores x 64 rows, weights
# replicated, zero inter-core communication (recurrence is sequential in
# time). Host pre-transposes/pre-scales weights and inputs.
#
# Per-core layout (b = 64 batch rows, R = 1024 reservoir, 8 chunks of 128):
#   sigmaT (state, transposed): SBUF [128, 8*64]; chunk ch holds
#     sigma[b, ch*128 + p] at [p, ch*64 + b]. Matmul stationary (lhsT).
#   preact PSUM [64, 512] per n-half, accumulated as
#     sum_ch sigmaT_ch.T @ W'_ch  +  x_t.T.T @ W_in   (u in fp16)
#   tanh on ScalarE (with free descale via activation input scale);
#   [b,n]->[r,b] chunk transposes on TensorE (identity matmul, the
#   cheapest executor measured: DMA-transpose and finer pipelining both
#   lost); fused leak+add on VectorE (scalar_tensor_tensor); fp8 state
#   copy cast on VectorE.
#
# Default variant 9 (= 7 + deeper pa/tbn buffering): the 16 recurrence matmuls run as 8 fp8-e4m3
# DoubleRow matmuls (two 128-row chunks contracted per pass, 2 fp8
# multiplies per PE cell per cycle). Scales keep fp8 in range: W' x512,
# sigma x16, W_in x8192 (fp16), descaled inside tanh. Measured ~25%
# faster than the all-fp16 variant 4; rel err 6.6e-3 vs 5.6e-4 (pass
# variant=4 to kernel() if a tighter tolerance is ever needed).
#
# Measured (paired min over reps, includes NEFF launch + input DMA):
#   v7 ~1.9 ms/exec vs all-fp16 v4 ~2.4 ms and bf16 v1 ~2.5 ms; fixed
#   per-exec overhead ~0.5 ms, so on-device step loop ~2.7 us/step.

import numpy as np
import ml_dtypes

BF16 = ml_dtypes.bfloat16

B = 512
T = 512
I = 64
R = 1024
O = 64
NCORES = 8
PB = B // NCORES  # 64 per-core batch rows
CH = R // 128     # 8 reservoir chunks

_prog_cache = {}


def _build_program(n_steps: int, variant: int = 2):
    if variant == 9:
        return _build_program_v9(n_steps)
    if variant == 8:
        return _build_program_v8(n_steps)
    if variant == 7:
        return _build_program_v7(n_steps)
    if variant == 6:
        return _build_program_v6(n_steps)
    if variant == 5:
        return _build_program_v5(n_steps)
    if variant == 4:
        return _build_program_v1(n_steps, fp16=True)
    if variant == 3:
        return _build_program_v3(n_steps)
    if variant == 2:
        return _build_program_v2(n_steps)
    return _build_program_v1(n_steps)


SW = 512.0   # fp8 weight scale
SS = 16.0    # fp8 sigma scale
SU = SW * SS  # combined preact scale (W_in pre-scaled by this)


def _build_program_v5(n_steps: int):
    """fp8 DoubleRow recurrence: chunk pairs contract 256 rows per pass.
    sigma stationary in e4m3 (x16), W' moving in e4m3 (x512) with pair-
    interleaved layout [p, q, h, n, 2]; u path stays fp16 with W_in
    pre-scaled by 8192; tanh descales via its input scale."""
    import concourse.bacc as bacc
    import concourse.mybir as mybir
    import concourse.tile as tile

    f32 = mybir.dt.float32
    fp16 = mybir.dt.float16
    fp8 = mybir.dt.float8e4
    AT = mybir.ActivationFunctionType
    ALU = mybir.AluOpType
    DR = mybir.MatmulPerfMode.DoubleRow

    from concourse.masks import make_identity

    nc = bacc.Bacc("TRN2", target_bir_lowering=False, debug=False)

    xt_d = nc.dram_tensor("xt", [I, n_steps * PB], fp16, kind="ExternalInput")
    wl8_d = nc.dram_tensor("wl8", [128, 4, 2, 2, 512], fp8, kind="ExternalInput")
    win_d = nc.dram_tensor("win", [I, R], fp16, kind="ExternalInput")
    wout_d = nc.dram_tensor("wout", [128, CH * O], fp16, kind="ExternalInput")
    y_d = nc.dram_tensor("y", [PB, O], f32, kind="ExternalOutput")

    with tile.TileContext(nc) as tc:
        with (
            tc.tile_pool(name="wpool", bufs=1) as wpool,
            tc.tile_pool(name="spool", bufs=1) as spool,
            tc.tile_pool(name="tpool", bufs=3) as tpool,
            tc.tile_pool(name="pa", bufs=4, space="PSUM") as pa_pool,
            tc.tile_pool(name="tp", bufs=2, space="PSUM") as tp_pool,
            tc.tile_pool(name="yp", bufs=1, space="PSUM") as yp_pool,
        ):
            xt_s = wpool.tile([I, n_steps * PB], fp16, tag="xt")
            wl8_s = wpool.tile([128, 4, 2, 2, 512], fp8, tag="wl8")
            win_s = wpool.tile([I, R], fp16, tag="win")
            wout_s = wpool.tile([128, CH * O], fp16, tag="wout")
            y_s = wpool.tile([PB, O], f32, tag="ys")

            nc.sync.dma_start(xt_s[:], xt_d[:])
            nc.sync.dma_start(wl8_s[:], wl8_d[:])
            nc.sync.dma_start(win_s[:], win_d[:])
            nc.sync.dma_start(wout_s[:], wout_d[:])
            ident = wpool.tile([64, 64], fp16, tag="ident")
            make_identity(nc, ident[:])

            sigF = [
                spool.tile([128, CH * PB], f32, tag=f"sigF{k}", name=f"sigF{k}")
                for k in range(2)
            ]
            sig8 = [
                spool.tile([128, CH * PB], fp8, tag=f"sig8{k}", name=f"sig8{k}")
                for k in range(2)
            ]
            sigB = spool.tile([128, CH * PB], fp16, tag="sigB")
            nc.vector.memset(sigF[0][:], 0.0)
            nc.vector.memset(sig8[0][:], 0.0)

            for t in range(n_steps):
                cur = t % 2
                nxt = (t + 1) % 2
                t_bn = tpool.tile([PB, R], fp16, tag="tbn")
                tp = tp_pool.tile([128, CH * PB], fp16, tag="tp")
                for h in (0, 1):
                    pa = pa_pool.tile([PB, 512], f32, tag="pa")
                    nc.tensor.matmul(
                        pa[:],
                        xt_s[:, t * PB : (t + 1) * PB],
                        win_s[:, h * 512 : (h + 1) * 512],
                        start=True,
                        stop=False,
                    )
                    for q in range(4):
                        pair = sig8[cur][:, q * 128 : (q + 1) * 128].rearrange(
                            "p (k b) -> p k b", k=2
                        )
                        nc.tensor.matmul(
                            pa[:],
                            pair,
                            wl8_s[:, q, h],
                            start=False,
                            stop=(q == 3),
                            perf_mode=DR,
                        )
                    nc.scalar.activation(
                        t_bn[:, h * 512 : (h + 1) * 512],
                        pa[:],
                        AT.Tanh,
                        scale=1.0 / SU,
                    )
                    for ch in range(4 * h, 4 * h + 4):
                        nc.tensor.transpose(
                            tp[:, ch * PB : (ch + 1) * PB],
                            t_bn[:, ch * 128 : (ch + 1) * 128],
                            ident[:],
                        )
                    sl = slice(h * 4 * PB, (h + 1) * 4 * PB)
                    nc.vector.scalar_tensor_tensor(
                        out=sigF[nxt][:, sl],
                        in0=sigF[cur][:, sl],
                        scalar=0.5,
                        in1=tp[:, sl],
                        op0=ALU.mult,
                        op1=ALU.add,
                    )
                    nc.scalar.mul(sig8[nxt][:, sl], sigF[nxt][:, sl], SS)

            fin = n_steps % 2
            nc.vector.tensor_copy(sigB[:], sigF[fin][:])
            yp = yp_pool.tile([PB, O], f32, tag="yp")
            for ch in range(CH):
                nc.tensor.matmul(
                    yp[:],
                    sigB[:, ch * PB : (ch + 1) * PB],
                    wout_s[:, ch * O : (ch + 1) * O],
                    start=(ch == 0),
                    stop=(ch == CH - 1),
                )
            nc.scalar.copy(y_s[:], yp[:])
            nc.sync.dma_start(y_d[:], y_s[:])

    nc.compile()
    return nc



def _build_program_v8(n_steps: int):
    """fp8 DoubleRow recurrence: chunk pairs contract 256 rows per pass.
    sigma stationary in e4m3 (x16), W' moving in e4m3 (x512) with pair-
    interleaved layout [p, q, h, n, 2]; u path stays fp16 with W_in
    pre-scaled by 8192; tanh descales via its input scale."""
    import concourse.bacc as bacc
    import concourse.mybir as mybir
    import concourse.tile as tile

    f32 = mybir.dt.float32
    fp16 = mybir.dt.float16
    fp8 = mybir.dt.float8e4
    AT = mybir.ActivationFunctionType
    ALU = mybir.AluOpType
    DR = mybir.MatmulPerfMode.DoubleRow

    from concourse.masks import make_identity

    nc = bacc.Bacc("TRN2", target_bir_lowering=False, debug=False)

    xt_d = nc.dram_tensor("xt", [I, n_steps * PB], fp16, kind="ExternalInput")
    wl8_d = nc.dram_tensor("wl8", [128, 4, 2, 1024], fp8, kind="ExternalInput")
    win_d = nc.dram_tensor("win", [I, R], fp16, kind="ExternalInput")
    wout_d = nc.dram_tensor("wout", [128, CH * O], fp16, kind="ExternalInput")
    y_d = nc.dram_tensor("y", [PB, O], f32, kind="ExternalOutput")

    with tile.TileContext(nc) as tc:
        with (
            tc.tile_pool(name="wpool", bufs=1) as wpool,
            tc.tile_pool(name="spool", bufs=1) as spool,
            tc.tile_pool(name="tpool", bufs=3) as tpool,
            tc.tile_pool(name="pa", bufs=4, space="PSUM") as pa_pool,
            tc.tile_pool(name="tp", bufs=2, space="PSUM") as tp_pool,
            tc.tile_pool(name="yp", bufs=1, space="PSUM") as yp_pool,
        ):
            xt_s = wpool.tile([I, n_steps * PB], fp16, tag="xt")
            wl8_s = wpool.tile([128, 4, 2, 1024], fp8, tag="wl8")
            win_s = wpool.tile([I, R], fp16, tag="win")
            wout_s = wpool.tile([128, CH * O], fp16, tag="wout")
            y_s = wpool.tile([PB, O], f32, tag="ys")

            nc.sync.dma_start(xt_s[:], xt_d[:])
            nc.sync.dma_start(wl8_s[:], wl8_d[:])
            nc.sync.dma_start(win_s[:], win_d[:])
            nc.sync.dma_start(wout_s[:], wout_d[:])
            ident = wpool.tile([64, 64], fp16, tag="ident")
            make_identity(nc, ident[:])

            sigF = [
                spool.tile([128, CH * PB], f32, tag=f"sigF{k}", name=f"sigF{k}")
                for k in range(2)
            ]
            sig8 = [
                spool.tile([128, CH * PB], fp8, tag=f"sig8{k}", name=f"sig8{k}")
                for k in range(2)
            ]
            sigB = spool.tile([128, CH * PB], fp16, tag="sigB")
            nc.vector.memset(sigF[0][:], 0.0)
            nc.vector.memset(sig8[0][:], 0.0)

            for t in range(n_steps):
                cur = t % 2
                nxt = (t + 1) % 2
                t_bn = tpool.tile([PB, R], fp16, tag="tbn")
                tp = tp_pool.tile([128, CH * PB], fp16, tag="tp")
                pa = pa_pool.tile([PB, R], fp16, tag="pa")
                nc.tensor.matmul(
                    pa[:],
                    xt_s[:, t * PB : (t + 1) * PB],
                    win_s[:],
                    start=True,
                    stop=False,
                )
                for q in range(4):
                    pair = sig8[cur][:, q * 128 : (q + 1) * 128].rearrange(
                        "p (k b) -> p k b", k=2
                    )
                    nc.tensor.matmul(
                        pa[:],
                        pair,
                        wl8_s[:, q],
                        start=False,
                        stop=(q == 3),
                        perf_mode=DR,
                    )
                for h in (0, 1):
                    nc.scalar.activation(
                        t_bn[:, h * 512 : (h + 1) * 512],
                        pa[:, h * 512 : (h + 1) * 512],
                        AT.Tanh,
                        scale=1.0 / 1024.0,
                    )
                    for ch in range(4 * h, 4 * h + 4):
                        nc.tensor.transpose(
                            tp[:, ch * PB : (ch + 1) * PB],
                            t_bn[:, ch * 128 : (ch + 1) * 128],
                            ident[:],
                        )
                    sl = slice(h * 4 * PB, (h + 1) * 4 * PB)
                    nc.vector.scalar_tensor_tensor(
                        out=sigF[nxt][:, sl],
                        in0=sigF[cur][:, sl],
                        scalar=0.5,
                        in1=tp[:, sl],
                        op0=ALU.mult,
                        op1=ALU.add,
                    )
                    nc.vector.tensor_scalar_mul(sig8[nxt][:, sl], sigF[nxt][:, sl], 2.0)

            fin = n_steps % 2
            nc.vector.tensor_copy(sigB[:], sigF[fin][:])
            yp = yp_pool.tile([PB, O], f32, tag="yp")
            for ch in range(CH):
                nc.tensor.matmul(
                    yp[:],
                    sigB[:, ch * PB : (ch + 1) * PB],
                    wout_s[:, ch * O : (ch + 1) * O],
                    start=(ch == 0),
                    stop=(ch == CH - 1),
                )
            nc.scalar.copy(y_s[:], yp[:])
            nc.sync.dma_start(y_d[:], y_s[:])

    nc.compile()
    return nc



def _build_program_v9(n_steps: int):
    """fp8 DoubleRow recurrence: chunk pairs contract 256 rows per pass.
    sigma stationary in e4m3 (x16), W' moving in e4m3 (x512) with pair-
    interleaved layout [p, q, h, n, 2]; u path stays fp16 with W_in
    pre-scaled by 8192; tanh descales via its input scale."""
    import concourse.bacc as bacc
    import concourse.mybir as mybir
    import concourse.tile as tile

    f32 = mybir.dt.float32
    fp16 = mybir.dt.float16
    fp8 = mybir.dt.float8e4
    AT = mybir.ActivationFunctionType
    ALU = mybir.AluOpType
    DR = mybir.MatmulPerfMode.DoubleRow

    from concourse.masks import make_identity

    nc = bacc.Bacc("TRN2", target_bir_lowering=False, debug=False)

    xt_d = nc.dram_tensor("xt", [I, n_steps * PB], fp16, kind="ExternalInput")
    wl8_d = nc.dram_tensor("wl8", [128, 4, 2, 2, 512], fp8, kind="ExternalInput")
    win_d = nc.dram_tensor("win", [I, R], fp16, kind="ExternalInput")
    wout_d = nc.dram_tensor("wout", [128, CH * O], fp16, kind="ExternalInput")
    y_d = nc.dram_tensor("y", [PB, O], f32, kind="ExternalOutput")

    with tile.TileContext(nc) as tc:
        with (
            tc.tile_pool(name="wpool", bufs=1) as wpool,
            tc.tile_pool(name="spool", bufs=1) as spool,
            tc.tile_pool(name="tpool", bufs=4) as tpool,
            tc.tile_pool(name="pa", bufs=5, space="PSUM") as pa_pool,
            tc.tile_pool(name="tp", bufs=2, space="PSUM") as tp_pool,
            tc.tile_pool(name="yp", bufs=1, space="PSUM") as yp_pool,
        ):
            xt_s = wpool.tile([I, n_steps * PB], fp16, tag="xt")
            wl8_s = wpool.tile([128, 4, 2, 2, 512], fp8, tag="wl8")
            win_s = wpool.tile([I, R], fp16, tag="win")
            wout_s = wpool.tile([128, CH * O], fp16, tag="wout")
            y_s = wpool.tile([PB, O], f32, tag="ys")

            nc.sync.dma_start(xt_s[:], xt_d[:])
            nc.sync.dma_start(wl8_s[:], wl8_d[:])
            nc.sync.dma_start(win_s[:], win_d[:])
            nc.sync.dma_start(wout_s[:], wout_d[:])
            ident = wpool.tile([64, 64], fp16, tag="ident")
            make_identity(nc, ident[:])

            sigF = [
                spool.tile([128, CH * PB], f32, tag=f"sigF{k}", name=f"sigF{k}")
                for k in range(2)
            ]
            sig8 = [
                spool.tile([128, CH * PB], fp8, tag=f"sig8{k}", name=f"sig8{k}")
                for k in range(2)
            ]
            sigB = spool.tile([128, CH * PB], fp16, tag="sigB")
            nc.vector.memset(sigF[0][:], 0.0)
            nc.vector.memset(sig8[0][:], 0.0)

            for t in range(n_steps):
                cur = t % 2
                nxt = (t + 1) % 2
                t_bn = tpool.tile([PB, R], fp16, tag="tbn")
                tp = tp_pool.tile([128, CH * PB], fp16, tag="tp")
                for h in (0, 1):
                    pa = pa_pool.tile([PB, 512], f32, tag="pa")
                    nc.tensor.matmul(
                        pa[:],
                        xt_s[:, t * PB : (t + 1) * PB],
                        win_s[:, h * 512 : (h + 1) * 512],
                        start=True,
                        stop=False,
                    )
                    for q in range(4):
                        pair = sig8[cur][:, q * 128 : (q + 1) * 128].rearrange(
                            "p (k b) -> p k b", k=2
                        )
                        nc.tensor.matmul(
                            pa[:],
                            pair,
                            wl8_s[:, q, h],
                            start=False,
                            stop=(q == 3),
                            perf_mode=DR,
                        )
                    nc.scalar.activation(
                        t_bn[:, h * 512 : (h + 1) * 512],
                        pa[:],
                        AT.Tanh,
                        scale=1.0 / SU,
                    )
                    for ch in range(4 * h, 4 * h + 4):
                        nc.tensor.transpose(
                            tp[:, ch * PB : (ch + 1) * PB],
                            t_bn[:, ch * 128 : (ch + 1) * 128],
                            ident[:],
                        )
                    sl = slice(h * 4 * PB, (h + 1) * 4 * PB)
                    nc.vector.scalar_tensor_tensor(
                        out=sigF[nxt][:, sl],
                        in0=sigF[cur][:, sl],
                        scalar=0.5,
                        in1=tp[:, sl],
                        op0=ALU.mult,
                        op1=ALU.add,
                    )
                    nc.vector.tensor_scalar_mul(sig8[nxt][:, sl], sigF[nxt][:, sl], SS)

            fin = n_steps % 2
            nc.vector.tensor_copy(sigB[:], sigF[fin][:])
            yp = yp_pool.tile([PB, O], f32, tag="yp")
            for ch in range(CH):
                nc.tensor.matmul(
                    yp[:],
                    sigB[:, ch * PB : (ch + 1) * PB],
                    wout_s[:, ch * O : (ch + 1) * O],
                    start=(ch == 0),
                    stop=(ch == CH - 1),
                )
            nc.scalar.copy(y_s[:], yp[:])
            nc.sync.dma_start(y_d[:], y_s[:])

    nc.compile()
    return nc



def _build_program_v7(n_steps: int):
    """fp8 DoubleRow recurrence: chunk pairs contract 256 rows per pass.
    sigma stationary in e4m3 (x16), W' moving in e4m3 (x512) with pair-
    interleaved layout [p, q, h, n, 2]; u path stays fp16 with W_in
    pre-scaled by 8192; tanh descales via its input scale."""
    import concourse.bacc as bacc
    import concourse.mybir as mybir
    import concourse.tile as tile

    f32 = mybir.dt.float32
    fp16 = mybir.dt.float16
    fp8 = mybir.dt.float8e4
    AT = mybir.ActivationFunctionType
    ALU = mybir.AluOpType
    DR = mybir.MatmulPerfMode.DoubleRow

    from concourse.masks import make_identity

    nc = bacc.Bacc("TRN2", target_bir_lowering=False, debug=False)

    xt_d = nc.dram_tensor("xt", [I, n_steps * PB], fp16, kind="ExternalInput")
    wl8_d = nc.dram_tensor("wl8", [128, 4, 2, 2, 512], fp8, kind="ExternalInput")
    win_d = nc.dram_tensor("win", [I, R], fp16, kind="ExternalInput")
    wout_d = nc.dram_tensor("wout", [128, CH * O], fp16, kind="ExternalInput")
    y_d = nc.dram_tensor("y", [PB, O], f32, kind="ExternalOutput")

    with tile.TileContext(nc) as tc:
        with (
            tc.tile_pool(name="wpool", bufs=1) as wpool,
            tc.tile_pool(name="spool", bufs=1) as spool,
            tc.tile_pool(name="tpool", bufs=3) as tpool,
            tc.tile_pool(name="pa", bufs=4, space="PSUM") as pa_pool,
            tc.tile_pool(name="tp", bufs=2, space="PSUM") as tp_pool,
            tc.tile_pool(name="yp", bufs=1, space="PSUM") as yp_pool,
        ):
            xt_s = wpool.tile([I, n_steps * PB], fp16, tag="xt")
            wl8_s = wpool.tile([128, 4, 2, 2, 512], fp8, tag="wl8")
            win_s = wpool.tile([I, R], fp16, tag="win")
            wout_s = wpool.tile([128, CH * O], fp16, tag="wout")
            y_s = wpool.tile([PB, O], f32, tag="ys")

            nc.sync.dma_start(xt_s[:], xt_d[:])
            nc.sync.dma_start(wl8_s[:], wl8_d[:])
            nc.sync.dma_start(win_s[:], win_d[:])
            nc.sync.dma_start(wout_s[:], wout_d[:])
            ident = wpool.tile([64, 64], fp16, tag="ident")
            make_identity(nc, ident[:])

            sigF = [
                spool.tile([128, CH * PB], f32, tag=f"sigF{k}", name=f"sigF{k}")
                for k in range(2)
            ]
            sig8 = [
                spool.tile([128, CH * PB], fp8, tag=f"sig8{k}", name=f"sig8{k}")
                for k in range(2)
            ]
            sigB = spool.tile([128, CH * PB], fp16, tag="sigB")
            nc.vector.memset(sigF[0][:], 0.0)
            nc.vector.memset(sig8[0][:], 0.0)

            for t in range(n_steps):
                cur = t % 2
                nxt = (t + 1) % 2
                t_bn = tpool.tile([PB, R], fp16, tag="tbn")
                tp = tp_pool.tile([128, CH * PB], fp16, tag="tp")
                for h in (0, 1):
                    pa = pa_pool.tile([PB, 512], f32, tag="pa")
                    nc.tensor.matmul(
                        pa[:],
                        xt_s[:, t * PB : (t + 1) * PB],
                        win_s[:, h * 512 : (h + 1) * 512],
                        start=True,
                        stop=False,
                    )
                    for q in range(4):
                        pair = sig8[cur][:, q * 128 : (q + 1) * 128].rearrange(
                            "p (k b) -> p k b", k=2
                        )
                        nc.tensor.matmul(
                            pa[:],
                            pair,
                            wl8_s[:, q, h],
                            start=False,
                            stop=(q == 3),
                            perf_mode=DR,
                        )
                    nc.scalar.activation(
                        t_bn[:, h * 512 : (h + 1) * 512],
                        pa[:],
                        AT.Tanh,
                        scale=1.0 / SU,
                    )
                    for ch in range(4 * h, 4 * h + 4):
                        nc.tensor.transpose(
                            tp[:, ch * PB : (ch + 1) * PB],
                            t_bn[:, ch * 128 : (ch + 1) * 128],
                            ident[:],
                        )
                    sl = slice(h * 4 * PB, (h + 1) * 4 * PB)
                    nc.vector.scalar_tensor_tensor(
                        out=sigF[nxt][:, sl],
                        in0=sigF[cur][:, sl],
                        scalar=0.5,
                        in1=tp[:, sl],
                        op0=ALU.mult,
                        op1=ALU.add,
                    )
                    nc.vector.tensor_scalar_mul(sig8[nxt][:, sl], sigF[nxt][:, sl], SS)

            fin = n_steps % 2
            nc.vector.tensor_copy(sigB[:], sigF[fin][:])
            yp = yp_pool.tile([PB, O], f32, tag="yp")
            for ch in range(CH):
                nc.tensor.matmul(
                    yp[:],
                    sigB[:, ch * PB : (ch + 1) * PB],
                    wout_s[:, ch * O : (ch + 1) * O],
                    start=(ch == 0),
                    stop=(ch == CH - 1),
                )
            nc.scalar.copy(y_s[:], yp[:])
            nc.sync.dma_start(y_d[:], y_s[:])

    nc.compile()
    return nc



def _build_program_v6(n_steps: int):
    """v5 + quarter-granular post-chain: tanh/blend/cast run per [*, 256]
    quarter so the last chunk's serial tail after the h=1 matmuls is
    shorter and the next step's matmuls start sooner."""
    import concourse.bacc as bacc
    import concourse.mybir as mybir
    import concourse.tile as tile
    from concourse.masks import make_identity

    f32 = mybir.dt.float32
    fp16 = mybir.dt.float16
    fp8 = mybir.dt.float8e4
    AT = mybir.ActivationFunctionType
    ALU = mybir.AluOpType
    DR = mybir.MatmulPerfMode.DoubleRow

    nc = bacc.Bacc("TRN2", target_bir_lowering=False, debug=False)

    xt_d = nc.dram_tensor("xt", [I, n_steps * PB], fp16, kind="ExternalInput")
    wl8_d = nc.dram_tensor("wl8", [128, 4, 2, 2, 512], fp8, kind="ExternalInput")
    win_d = nc.dram_tensor("win", [I, R], fp16, kind="ExternalInput")
    wout_d = nc.dram_tensor("wout", [128, CH * O], fp16, kind="ExternalInput")
    y_d = nc.dram_tensor("y", [PB, O], f32, kind="ExternalOutput")

    with tile.TileContext(nc) as tc:
        with (
            tc.tile_pool(name="wpool", bufs=1) as wpool,
            tc.tile_pool(name="spool", bufs=1) as spool,
            tc.tile_pool(name="tpool", bufs=4) as tpool,
            tc.tile_pool(name="pa", bufs=4, space="PSUM") as pa_pool,
            tc.tile_pool(name="tp", bufs=2, space="PSUM") as tp_pool,
            tc.tile_pool(name="yp", bufs=1, space="PSUM") as yp_pool,
        ):
            xt_s = wpool.tile([I, n_steps * PB], fp16, tag="xt")
            wl8_s = wpool.tile([128, 4, 2, 2, 512], fp8, tag="wl8")
            win_s = wpool.tile([I, R], fp16, tag="win")
            wout_s = wpool.tile([128, CH * O], fp16, tag="wout")
            y_s = wpool.tile([PB, O], f32, tag="ys")

            nc.sync.dma_start(xt_s[:], xt_d[:])
            nc.sync.dma_start(wl8_s[:], wl8_d[:])
            nc.sync.dma_start(win_s[:], win_d[:])
            nc.sync.dma_start(wout_s[:], wout_d[:])
            ident = wpool.tile([64, 64], fp16, tag="ident")
            make_identity(nc, ident[:])

            sigF = [
                spool.tile([128, CH * PB], f32, tag=f"sigF{k}", name=f"sigF{k}")
                for k in range(2)
            ]
            sig8 = [
                spool.tile([128, CH * PB], fp8, tag=f"sig8{k}", name=f"sig8{k}")
                for k in range(2)
            ]
            sigB = spool.tile([128, CH * PB], fp16, tag="sigB")
            nc.vector.memset(sigF[0][:], 0.0)
            nc.vector.memset(sig8[0][:], 0.0)

            for t in range(n_steps):
                cur = t % 2
                nxt = (t + 1) % 2
                t_bn = tpool.tile([PB, R], fp16, tag="tbn")
                tp = tp_pool.tile([128, CH * PB], fp16, tag="tp")
                for h in (0, 1):
                    pa = pa_pool.tile([PB, 512], f32, tag="pa")
                    nc.tensor.matmul(
                        pa[:],
                        xt_s[:, t * PB : (t + 1) * PB],
                        win_s[:, h * 512 : (h + 1) * 512],
                        start=True,
                        stop=False,
                    )
                    for q in range(4):
                        pair = sig8[cur][:, q * 128 : (q + 1) * 128].rearrange(
                            "p (k b) -> p k b", k=2
                        )
                        nc.tensor.matmul(
                            pa[:],
                            pair,
                            wl8_s[:, q, h],
                            start=False,
                            stop=(q == 3),
                            perf_mode=DR,
                        )
                    for q2 in (0, 1):
                        nsl = slice(h * 512 + q2 * 256, h * 512 + q2 * 256 + 256)
                        nc.scalar.activation(
                            t_bn[:, nsl], pa[:, q2 * 256 : (q2 + 1) * 256],
                            AT.Tanh, scale=1.0 / SU,
                        )
                        c0 = 4 * h + 2 * q2
                        for ch in (c0, c0 + 1):
                            nc.tensor.transpose(
                                tp[:, ch * PB : (ch + 1) * PB],
                                t_bn[:, ch * 128 : (ch + 1) * 128],
                                ident[:],
                            )
                        sl = slice(c0 * PB, (c0 + 2) * PB)
                        nc.vector.scalar_tensor_tensor(
                            out=sigF[nxt][:, sl],
                            in0=sigF[cur][:, sl],
                            scalar=0.5,
                            in1=tp[:, sl],
                            op0=ALU.mult,
                            op1=ALU.add,
                        )
                        nc.scalar.mul(sig8[nxt][:, sl], sigF[nxt][:, sl], SS)

            fin = n_steps % 2
            nc.vector.tensor_copy(sigB[:], sigF[fin][:])
            yp = yp_pool.tile([PB, O], f32, tag="yp")
            for ch in range(CH):
                nc.tensor.matmul(
                    yp[:],
                    sigB[:, ch * PB : (ch + 1) * PB],
                    wout_s[:, ch * O : (ch + 1) * O],
                    start=(ch == 0),
                    stop=(ch == CH - 1),
                )
            nc.scalar.copy(y_s[:], yp[:])
            nc.sync.dma_start(y_d[:], y_s[:])

    nc.compile()
    return nc


def _build_program_v3(n_steps: int):
    """v1 layout, but the per-step [b,n]->[r,b] transposes run on the DMA
    engines (HW transpose mode, bf16) instead of TensorE, and the leak+add
    runs per chunk so each chunk's chain starts as soon as its DMA lands."""
    import concourse.bacc as bacc
    import concourse.mybir as mybir
    import concourse.tile as tile

    f32 = mybir.dt.float32
    bf16 = mybir.dt.bfloat16
    AT = mybir.ActivationFunctionType
    ALU = mybir.AluOpType

    nc = bacc.Bacc("TRN2", target_bir_lowering=False, debug=False)

    xt_d = nc.dram_tensor("xt", [I, n_steps * PB], bf16, kind="ExternalInput")
    wl_d = nc.dram_tensor("wl", [128, CH * R], bf16, kind="ExternalInput")
    win_d = nc.dram_tensor("win", [I, R], bf16, kind="ExternalInput")
    wout_d = nc.dram_tensor("wout", [128, CH * O], bf16, kind="ExternalInput")
    y_d = nc.dram_tensor("y", [PB, O], f32, kind="ExternalOutput")

    with tile.TileContext(nc) as tc:
        with (
            tc.tile_pool(name="wpool", bufs=1) as wpool,
            tc.tile_pool(name="spool", bufs=1) as spool,
            tc.tile_pool(name="tpool", bufs=3) as tpool,
            tc.tile_pool(name="pa", bufs=4, space="PSUM") as pa_pool,
            tc.tile_pool(name="yp", bufs=1, space="PSUM") as yp_pool,
        ):
            xt_s = wpool.tile([I, n_steps * PB], bf16, tag="xt")
            wl_s = wpool.tile([128, CH * R], bf16, tag="wl")
            win_s = wpool.tile([I, R], bf16, tag="win")
            wout_s = wpool.tile([128, CH * O], bf16, tag="wout")
            y_s = wpool.tile([PB, O], f32, tag="ys")

            nc.sync.dma_start(xt_s[:], xt_d[:])
            nc.sync.dma_start(wl_s[:], wl_d[:])
            nc.sync.dma_start(win_s[:], win_d[:])
            nc.sync.dma_start(wout_s[:], wout_d[:])

            sigF = [
                spool.tile([128, CH * PB], f32, tag=f"sigF{k}", name=f"sigF{k}")
                for k in range(2)
            ]
            sigB = [
                spool.tile([128, CH * PB], bf16, tag=f"sigB{k}", name=f"sigB{k}")
                for k in range(2)
            ]
            nc.vector.memset(sigF[0][:], 0.0)
            nc.vector.memset(sigB[0][:], 0.0)

            for t in range(n_steps):
                cur = t % 2
                nxt = (t + 1) % 2
                t_bn = tpool.tile([PB, R], bf16, tag="tbn")
                tp = tpool.tile([128, CH * PB], bf16, tag="tp")
                for h in (0, 1):
                    pa = pa_pool.tile([PB, 512], f32, tag="pa")
                    nc.tensor.matmul(
                        pa[:],
                        xt_s[:, t * PB : (t + 1) * PB],
                        win_s[:, h * 512 : (h + 1) * 512],
                        start=True,
                        stop=False,
                    )
                    for ch in range(CH):
                        nc.tensor.matmul(
                            pa[:],
                            sigB[cur][:, ch * PB : (ch + 1) * PB],
                            wl_s[:, ch * R + h * 512 : ch * R + h * 512 + 512],
                            start=False,
                            stop=(ch == CH - 1),
                        )
                    nc.scalar.activation(
                        t_bn[:, h * 512 : (h + 1) * 512], pa[:], AT.Tanh
                    )
                    for ch in range(4 * h, 4 * h + 4):
                        nc.sync.dma_start(
                            out=tp[:, ch * PB : (ch + 1) * PB],
                            in_=t_bn[:, ch * 128 : (ch + 1) * 128],
                            transpose=True,
                        )
                    sl = slice(h * 4 * PB, (h + 1) * 4 * PB)
                    nc.vector.scalar_tensor_tensor(
                        out=sigF[nxt][:, sl],
                        in0=sigF[cur][:, sl],
                        scalar=0.5,
                        in1=tp[:, sl],
                        op0=ALU.mult,
                        op1=ALU.add,
                    )
                    nc.vector.tensor_copy(sigB[nxt][:, sl], sigF[nxt][:, sl])

            fin = n_steps % 2
            yp = yp_pool.tile([PB, O], f32, tag="yp")
            for ch in range(CH):
                nc.tensor.matmul(
                    yp[:],
                    sigB[fin][:, ch * PB : (ch + 1) * PB],
                    wout_s[:, ch * O : (ch + 1) * O],
                    start=(ch == 0),
                    stop=(ch == CH - 1),
                )
            nc.scalar.copy(y_s[:], yp[:])
            nc.sync.dma_start(y_d[:], y_s[:])

    nc.compile()
    return nc


def _build_program_v2(n_steps: int):
    """Column-paired variant: for each reservoir chunk, the two n-halves run
    as two concurrent 64-col stationary groups (tile_position (0,0)/(0,64)),
    so the 128x128 PE array is fully used. preact PSUM is [128, 512]:
    rows 0:64 = n 0:511, rows 64:128 = n 512:1023 (same batch rows)."""
    import concourse.bacc as bacc
    import concourse.mybir as mybir
    import concourse.tile as tile

    f32 = mybir.dt.float32
    bf16 = mybir.dt.bfloat16
    AT = mybir.ActivationFunctionType
    ALU = mybir.AluOpType

    nc = bacc.Bacc("TRN2", target_bir_lowering=False, debug=False)

    xt_d = nc.dram_tensor("xt", [I, n_steps * PB], bf16, kind="ExternalInput")
    wl_d = nc.dram_tensor("wl", [128, CH * R], bf16, kind="ExternalInput")
    win_d = nc.dram_tensor("win", [I, R], bf16, kind="ExternalInput")
    wout_d = nc.dram_tensor("wout", [128, CH * O], bf16, kind="ExternalInput")
    id_d = nc.dram_tensor("ident", [128, 64], bf16, kind="ExternalInput")
    y_d = nc.dram_tensor("y", [PB, O], f32, kind="ExternalOutput")

    with tile.TileContext(nc) as tc:
        with (
            tc.tile_pool(name="wpool", bufs=1) as wpool,
            tc.tile_pool(name="spool", bufs=1) as spool,
            tc.tile_pool(name="tpool", bufs=3) as tpool,
            tc.tile_pool(name="pa", bufs=3, space="PSUM") as pa_pool,
            tc.tile_pool(name="tp", bufs=2, space="PSUM") as tp_pool,
            tc.tile_pool(name="yp", bufs=1, space="PSUM") as yp_pool,
        ):
            xt_s = wpool.tile([I, n_steps * PB], bf16, tag="xt")
            wl_s = wpool.tile([128, CH * R], bf16, tag="wl")
            win_s = wpool.tile([I, R], bf16, tag="win")
            wout_s = wpool.tile([128, CH * O], bf16, tag="wout")
            ident = wpool.tile([128, 64], bf16, tag="ident")
            y_s = wpool.tile([PB, O], f32, tag="ys")

            nc.sync.dma_start(xt_s[:], xt_d[:])
            nc.sync.dma_start(wl_s[:], wl_d[:])
            nc.sync.dma_start(win_s[:], win_d[:])
            nc.sync.dma_start(wout_s[:], wout_d[:])
            nc.sync.dma_start(ident[:], id_d[:])

            sigF = [
                spool.tile([128, CH * PB], f32, tag=f"sigF{k}", name=f"sigF{k}")
                for k in range(2)
            ]
            sigB = [
                spool.tile([128, CH * PB], bf16, tag=f"sigB{k}", name=f"sigB{k}")
                for k in range(2)
            ]
            nc.vector.memset(sigF[0][:], 0.0)
            nc.vector.memset(sigB[0][:], 0.0)

            for t in range(n_steps):
                cur = t % 2
                nxt = (t + 1) % 2
                t_bn = tpool.tile([128, 512], bf16, tag="tbn")
                tp = tp_pool.tile([128, CH * PB], bf16, tag="tp")
                pa = pa_pool.tile([128, 512], f32, tag="pa")
                # input projection pair (K=64)
                xsl = xt_s[:, t * PB : (t + 1) * PB]
                nc.tensor.matmul(
                    pa[0:64, :], xsl, win_s[:, 0:512],
                    start=True, stop=False, tile_position=(0, 0),
                )
                nc.tensor.matmul(
                    pa[64:128, :], xsl, win_s[:, 512:1024],
                    start=True, stop=False, tile_position=(0, 64),
                )
                # recurrence pairs (K=128)
                for ch in range(CH):
                    ssl = sigB[cur][:, ch * PB : (ch + 1) * PB]
                    last = ch == CH - 1
                    nc.tensor.matmul(
                        pa[0:64, :], ssl,
                        wl_s[:, ch * R : ch * R + 512],
                        start=False, stop=last, tile_position=(0, 0),
                    )
                    nc.tensor.matmul(
                        pa[64:128, :], ssl,
                        wl_s[:, ch * R + 512 : ch * R + 1024],
                        start=False, stop=last, tile_position=(0, 64),
                    )
                # tanh per half (lane-aligned in/out)
                nc.scalar.activation(t_bn[0:64, :], pa[0:64, :], AT.Tanh)
                nc.scalar.activation(t_bn[64:128, :], pa[64:128, :], AT.Tanh)
                # transpose chunks back to [r, b]; chunks 4-7 live on rows 64:128
                for ch in range(CH):
                    base = 0 if ch < 4 else 64
                    col = (ch % 4) * 128
                    nc.tensor.transpose(
                        tp[:, ch * PB : (ch + 1) * PB],
                        t_bn[base : base + 64, col : col + 128],
                        ident[base : base + 64, :],
                    )
                # fused leak+add and bf16 copy per half
                for h in (0, 1):
                    sl = slice(h * 4 * PB, (h + 1) * 4 * PB)
                    nc.vector.scalar_tensor_tensor(
                        out=sigF[nxt][:, sl],
                        in0=sigF[cur][:, sl],
                        scalar=0.5,
                        in1=tp[:, sl],
                        op0=ALU.mult,
                        op1=ALU.add,
                    )
                    nc.vector.tensor_copy(sigB[nxt][:, sl], sigF[nxt][:, sl])

            fin = n_steps % 2
            yp = yp_pool.tile([PB, O], f32, tag="yp")
            for ch in range(CH):
                nc.tensor.matmul(
                    yp[:],
                    sigB[fin][:, ch * PB : (ch + 1) * PB],
                    wout_s[:, ch * O : (ch + 1) * O],
                    start=(ch == 0),
                    stop=(ch == CH - 1),
                )
            nc.scalar.copy(y_s[:], yp[:])
            nc.sync.dma_start(y_d[:], y_s[:])

    nc.compile()
    return nc


def _build_program_v1(n_steps: int, fp16: bool = False):
    import concourse.bacc as bacc
    import concourse.mybir as mybir
    import concourse.tile as tile
    from concourse.masks import make_identity

    f32 = mybir.dt.float32
    bf16 = mybir.dt.float16 if fp16 else mybir.dt.bfloat16
    AT = mybir.ActivationFunctionType
    ALU = mybir.AluOpType

    nc = bacc.Bacc("TRN2", target_bir_lowering=False, debug=False)

    xt_d = nc.dram_tensor("xt", [I, n_steps * PB], bf16, kind="ExternalInput")
    wl_d = nc.dram_tensor("wl", [128, CH * R], bf16, kind="ExternalInput")
    win_d = nc.dram_tensor("win", [I, R], bf16, kind="ExternalInput")
    wout_d = nc.dram_tensor("wout", [128, CH * O], bf16, kind="ExternalInput")
    y_d = nc.dram_tensor("y", [PB, O], f32, kind="ExternalOutput")

    with tile.TileContext(nc) as tc:
        with (
            tc.tile_pool(name="wpool", bufs=1) as wpool,
            tc.tile_pool(name="spool", bufs=1) as spool,
            tc.tile_pool(name="tpool", bufs=3) as tpool,
            tc.tile_pool(name="pa", bufs=4, space="PSUM") as pa_pool,
            tc.tile_pool(name="tp", bufs=2, space="PSUM") as tp_pool,
            tc.tile_pool(name="yp", bufs=1, space="PSUM") as yp_pool,
        ):
            xt_s = wpool.tile([I, n_steps * PB], bf16, tag="xt")
            wl_s = wpool.tile([128, CH * R], bf16, tag="wl")
            win_s = wpool.tile([I, R], bf16, tag="win")
            wout_s = wpool.tile([128, CH * O], bf16, tag="wout")
            ident = wpool.tile([64, 64], bf16, tag="ident")
            y_s = wpool.tile([PB, O], f32, tag="ys")

            nc.sync.dma_start(xt_s[:], xt_d[:])
            nc.sync.dma_start(wl_s[:], wl_d[:])
            nc.sync.dma_start(win_s[:], win_d[:])
            nc.sync.dma_start(wout_s[:], wout_d[:])
            make_identity(nc, ident[:])

            # ping-pong state, sigF = fp32 master, sigB = bf16 copy for PE
            sigF = [
                spool.tile([128, CH * PB], f32, tag=f"sigF{k}", name=f"sigF{k}")
                for k in range(2)
            ]
            sigB = [
                spool.tile([128, CH * PB], bf16, tag=f"sigB{k}", name=f"sigB{k}")
                for k in range(2)
            ]
            nc.vector.memset(sigF[0][:], 0.0)
            nc.vector.memset(sigB[0][:], 0.0)

            for t in range(n_steps):
                cur = t % 2
                nxt = (t + 1) % 2
                t_bn = tpool.tile([PB, R], bf16, tag="tbn")
                tp = tp_pool.tile([128, CH * PB], bf16, tag="tp")
                for h in (0, 1):
                    pa = pa_pool.tile([PB, 512], f32, tag="pa")
                    # input projection first: x always ready, keeps PE fed
                    nc.tensor.matmul(
                        pa[:],
                        xt_s[:, t * PB : (t + 1) * PB],
                        win_s[:, h * 512 : (h + 1) * 512],
                        start=True,
                        stop=False,
                    )
                    for ch in range(CH):
                        nc.tensor.matmul(
                            pa[:],
                            sigB[cur][:, ch * PB : (ch + 1) * PB],
                            wl_s[:, ch * R + h * 512 : ch * R + h * 512 + 512],
                            start=False,
                            stop=(ch == CH - 1),
                        )
                    nc.scalar.activation(t_bn[:, h * 512 : (h + 1) * 512], pa[:], AT.Tanh)
                    # transpose this half's 4 chunks back to [r, b] layout
                    for ch in range(4 * h, 4 * h + 4):
                        nc.tensor.transpose(
                            tp[:, ch * PB : (ch + 1) * PB],
                            t_bn[:, ch * 128 : (ch + 1) * 128],
                            ident[:],
                        )
                    # fused leak + add for this half's chunks, then bf16 copy
                    sl = slice(h * 4 * PB, (h + 1) * 4 * PB)
                    nc.vector.scalar_tensor_tensor(
                        out=sigF[nxt][:, sl],
                        in0=sigF[cur][:, sl],
                        scalar=0.5,
                        in1=tp[:, sl],
                        op0=ALU.mult,
                        op1=ALU.add,
                    )
                    nc.vector.tensor_copy(sigB[nxt][:, sl], sigF[nxt][:, sl])

            # output projection: y = sigma_T @ (0.5*W_out)
            fin = n_steps % 2
            yp = yp_pool.tile([PB, O], f32, tag="yp")
            for ch in range(CH):
                nc.tensor.matmul(
                    yp[:],
                    sigB[fin][:, ch * PB : (ch + 1) * PB],
                    wout_s[:, ch * O : (ch + 1) * O],
                    start=(ch == 0),
                    stop=(ch == CH - 1),
                )
            nc.scalar.copy(y_s[:], yp[:])
            nc.sync.dma_start(y_d[:], y_s[:])

    nc.compile()
    return nc


def _prep_inputs(input, W_reservoir, W_in, W_out, n_steps, variant=2):
    if variant == 8:
        return _prep_inputs_v8(input, W_reservoir, W_in, W_out, n_steps)
    if variant in (5, 6, 7, 9):
        return _prep_inputs_v5(input, W_reservoir, W_in, W_out, n_steps)
    lp = np.float16 if variant == 4 else BF16
    wl = (0.5 * W_reservoir).reshape(CH, 128, R).transpose(1, 0, 2).reshape(128, CH * R)
    wl = np.ascontiguousarray(wl, dtype=np.float32).astype(lp)
    win = np.ascontiguousarray(W_in, dtype=np.float32).astype(lp)
    wout = (0.5 * W_out).reshape(CH, 128, O).transpose(1, 0, 2).reshape(128, CH * O)
    wout = np.ascontiguousarray(wout, dtype=np.float32).astype(lp)

    ident = np.vstack([np.eye(64), np.eye(64)]).astype(BF16)  # [128, 64]

    in_maps = []
    for c in range(NCORES):
        xs = input[c * PB : (c + 1) * PB, :n_steps, :]  # [PB, n_steps, I]
        xt = np.ascontiguousarray(xs.transpose(2, 1, 0)).reshape(I, n_steps * PB)
        xt = xt.astype(lp)
        m = {"xt": xt, "wl": wl, "win": win, "wout": wout}
        if variant == 2:
            m["ident"] = ident
        in_maps.append(m)
    return in_maps


def _prep_inputs_v5(input, W_reservoir, W_in, W_out, n_steps):
    import concourse.mybir as mybir

    f8 = mybir.dt.np(mybir.dt.float8e4)
    # [q, j, p, n] -> [p, q, n, j] -> [128, 4, 2, 512, 2]
    # [q, j, p, (h n)] -> [p, q, j, h, n] -> [p, q, h, j, n]
    W2 = (0.5 * W_reservoir * SW).astype(np.float32).reshape(4, 2, 128, 2, 512)
    wl8 = np.ascontiguousarray(np.transpose(W2, (2, 0, 3, 1, 4)))  # [128,4,h,j,512]
    wl8 = wl8.astype(f8)
    win = np.ascontiguousarray(W_in * SU, dtype=np.float32).astype(np.float16)
    wout = (0.5 * W_out).reshape(CH, 128, O).transpose(1, 0, 2).reshape(128, CH * O)
    wout = np.ascontiguousarray(wout, dtype=np.float32).astype(np.float16)

    in_maps = []
    for c in range(NCORES):
        xs = input[c * PB : (c + 1) * PB, :n_steps, :]
        xt = np.ascontiguousarray(xs.transpose(2, 1, 0)).reshape(I, n_steps * PB)
        xt = xt.astype(np.float16)
        in_maps.append({"xt": xt, "wl8": wl8, "win": win, "wout": wout})
    return in_maps


def _prep_inputs_v8(input, W_reservoir, W_in, W_out, n_steps):
    import concourse.mybir as mybir

    f8 = mybir.dt.np(mybir.dt.float8e4)
    # v8 scales: sigma x2, W x512, combined 1024 (fits fp16 psum range)
    W2 = (0.5 * W_reservoir * SW).astype(np.float32).reshape(4, 2, 128, R)
    wl8 = np.ascontiguousarray(np.transpose(W2, (2, 0, 1, 3))).astype(f8)
    win = np.ascontiguousarray(W_in * (SW * 2.0), dtype=np.float32).astype(np.float16)
    wout = (0.5 * W_out).reshape(CH, 128, O).transpose(1, 0, 2).reshape(128, CH * O)
    wout = np.ascontiguousarray(wout, dtype=np.float32).astype(np.float16)

    in_maps = []
    for c in range(NCORES):
        xs = input[c * PB : (c + 1) * PB, :n_steps, :]
        xt = np.ascontiguousarray(xs.transpose(2, 1, 0)).reshape(I, n_steps * PB)
        xt = xt.astype(np.float16)
        in_maps.append({"xt": xt, "wl8": wl8, "win": win, "wout": wout})
    return in_maps


def kernel(input, W_reservoir, W_in, W_out, n_steps=T, trace=False, variant=9):
    from concourse.bass_utils import run_bass_kernel_spmd

    input = np.asarray(input, dtype=np.float32)
    W_reservoir = np.asarray(W_reservoir, dtype=np.float32)
    W_in = np.asarray(W_in, dtype=np.float32)
    W_out = np.asarray(W_out, dtype=np.float32)

    key = (n_steps, variant)
    if key not in _prog_cache:
        _prog_cache[key] = _build_program(n_steps, variant)
    nc = _prog_cache[key]

    in_maps = _prep_inputs(input, W_reservoir, W_in, W_out, n_steps, variant)
    res = run_bass_kernel_spmd(
        nc, in_maps, core_ids=list(range(NCORES)), trace=trace
    )
    out = np.empty((B, O), dtype=np.float32)
    for c in range(NCORES):
        out[c * PB : (c + 1) * PB] = res.results[c]["y"]
    if trace:
        kernel._last_results = res
    return out

